# revision 25
# baseline (speedup 1.0000x reference)
"""3x3 conv2d (stride 1, pad 1) over [32, 1024, 1024] fp32, data-parallel on 8 TRN2 cores.

Strategy (memory-bound regime):
  - Pure data parallel: each core gets 4 images; no collectives.
  - Host pads each image to [1026, 1026] with zeros and casts to bf16
    (halves the input DMA traffic; rel err ~1e-3 << 2e-2 gate).
  - On device, the 3x3 conv is computed with banded matmuls on the
    TensorEngine: rows go on the partition dim; the 3 row-taps become a
    banded lhsT [K=m+2, M=m] built on host from the runtime weight; the 3
    column-taps become 3 column-shifted matmuls accumulating in PSUM.
  - Per output tile of 126 rows x 512 cols: 3 matmuls (dv = -1, 0, +1),
    then DVE copies PSUM->SBUF, then DMA out as fp32. Input DMAs issue on
    the SP (sync) HWDGE queue, output DMAs on the ACT (scalar) queue so the
    two descriptor streams pipeline independently; lhsT is kept 128 columns
    wide (full array) so bf16 fast-weight-load stays enabled.

Measured (8 cores, steady state via in-NEFF For_i repeat slope): ~83 us per
kernel body vs ~74 us modeled DMA roofline (26 MB at ~360 GB/s/core).
"""

import numpy as np
import ml_dtypes

import concourse.bacc as bacc
import concourse.mybir as mybir
from concourse.tile import TileContext
from concourse.bass_utils import run_bass_kernel_spmd

B, H, W = 32, 1024, 1024
N_CORES = 8
B_LOC = B // N_CORES
M_TILE = 126  # output rows per tile; K = M + 2 input rows <= 128 partitions


def _build_nc(
    b_loc=B_LOC,
    h=H,
    w=W,
    bufs_x=8,
    bufs_ps=4,
    bufs_o=8,
    copy_engines=("vector",),
    out_dma_split=1,
    copy_full=False,
    psum_per_chunk=False,
    in_dma_engine="sync",
    out_dma_engine="scalar",
    repeat=1,
    lhst_full=True,
    dma_alternate=False,
):
    hp, wp = h + 2, w + 2
    n_row_tiles = (h + M_TILE - 1) // M_TILE
    n_col_chunks = (w + 511) // 512
    psum_w = min(w, 1024)

    nc = bacc.Bacc("TRN2")
    x_d = nc.dram_tensor("x", [b_loc, hp, wp], mybir.dt.bfloat16, kind="ExternalInput")
    w_d = nc.dram_tensor("wb", [128, 384], mybir.dt.bfloat16, kind="ExternalInput")
    o_d = nc.dram_tensor("out", [b_loc, h, w], mybir.dt.float32, kind="ExternalOutput")

    with TileContext(nc) as tc:
        with (
            tc.tile_pool(name="wpool", bufs=1) as wpool,
            tc.tile_pool(name="xpool", bufs=bufs_x) as xpool,
            tc.tile_pool(name="pspool", bufs=bufs_ps, space="PSUM") as pspool,
            tc.tile_pool(name="opool", bufs=bufs_o) as opool,
        ):
            in_dma = getattr(nc, in_dma_engine)
            out_dma = getattr(nc, out_dma_engine)
            wt = wpool.tile([128, 384], mybir.dt.bfloat16)
            nc.sync.dma_start(out=wt[:], in_=w_d[:, :])

            import contextlib

            rep_ctx = (
                tc.For_i(0, repeat, 1) if repeat > 1 else contextlib.nullcontext()
            )
            with rep_ctx:
                _emit_body(
                    nc, tc, b_loc, h, w, wp, n_row_tiles, n_col_chunks,
                    xpool, pspool, opool, wt, x_d, o_d,
                    copy_engines, out_dma_split, copy_full, psum_per_chunk,
                    in_dma, out_dma, psum_w, lhst_full, dma_alternate,
                )
    return nc


def _emit_body(
    nc, tc, b_loc, h, w, wp, n_row_tiles, n_col_chunks,
    xpool, pspool, opool, wt, x_d, o_d,
    copy_engines, out_dma_split, copy_full, psum_per_chunk,
    in_dma, out_dma, psum_w, lhst_full=False, dma_alternate=False,
):
    import concourse.mybir as mybir

    if True:  # keep indentation shallow
        if True:
            it = 0
            for img in range(b_loc):
                for t in range(n_row_tiles):
                    r0 = t * M_TILE
                    m = min(M_TILE, h - r0)
                    k = m + 2
                    if dma_alternate:
                        in_dma = (nc.sync, nc.scalar)[it % 2]
                        out_dma = (nc.scalar, nc.sync)[it % 2]
                    xt = xpool.tile([128, wp], mybir.dt.bfloat16)
                    in_dma.dma_start(out=xt[:k, :], in_=x_d[img, r0 : r0 + k, :])
                    if not psum_per_chunk:
                        ps = pspool.tile([128, psum_w], mybir.dt.float32)
                    ot = opool.tile([128, w], mybir.dt.float32)
                    for ci in range(n_col_chunks):
                        c0 = ci * 512
                        n = min(512, w - c0)
                        m_mm = 128 if lhst_full else m
                        if psum_per_chunk:
                            ps_c = pspool.tile(
                                [128, 512], mybir.dt.float32, name=f"psc_{it}_{ci}", tag="psc"
                            )
                            dst = ps_c[:m_mm, :n]
                            src = ps_c[:m, :n]
                        else:
                            dst = ps[:m_mm, c0 : c0 + n]
                            src = ps[:m, c0 : c0 + n]
                        for j, dv in enumerate((-1, 0, 1)):
                            nc.tensor.matmul(
                                dst,
                                lhsT=wt[:k, 128 * (dv + 1) : 128 * (dv + 1) + m_mm],
                                rhs=xt[:k, c0 + 1 + dv : c0 + 1 + dv + n],
                                start=(j == 0),
                                stop=(j == 2),
                            )
                        if not copy_full:
                            eng = copy_engines[
                                (it * n_col_chunks + ci) % len(copy_engines)
                            ]
                            if eng == "scalar":
                                nc.scalar.copy(ot[:m, c0 : c0 + n], src)
                            else:
                                nc.vector.tensor_copy(ot[:m, c0 : c0 + n], src)
                    if copy_full:
                        eng = copy_engines[it % len(copy_engines)]
                        if eng == "scalar":
                            nc.scalar.copy(ot[:m, :w], ps[:m, :w])
                        else:
                            nc.vector.tensor_copy(ot[:m, :w], ps[:m, :w])
                    if out_dma_split == 1:
                        out_dma.dma_start(out=o_d[img, r0 : r0 + m, :], in_=ot[:m, :w])
                    else:
                        step = w // out_dma_split
                        for s in range(out_dma_split):
                            out_dma.dma_start(
                                out=o_d[img, r0 : r0 + m, s * step : (s + 1) * step],
                                in_=ot[:m, s * step : (s + 1) * step],
                            )
                    it += 1


def _banded_weights(weight):
    """wb[k, 128*dvi + m] = weight[k - m, dvi] for 0 <= k - m <= 2, else 0."""
    wb = np.zeros((128, 384), np.float32)
    for dvi in range(3):
        blk = wb[:, 128 * dvi : 128 * dvi + 128]
        for d in range(3):
            rows = np.arange(d, 128)
            cols = np.arange(0, 128 - d)
            blk[rows, cols] = float(weight[d, dvi])
    return wb


def _prep_inputs(X, weight):
    X = np.asarray(X, dtype=np.float32)
    weight = np.asarray(weight, dtype=np.float32)
    Xp = np.zeros((X.shape[0], X.shape[1] + 2, X.shape[2] + 2), np.float32)
    Xp[:, 1:-1, 1:-1] = X
    Xb = Xp.astype(ml_dtypes.bfloat16)
    wb = _banded_weights(weight).astype(ml_dtypes.bfloat16)
    return Xb, wb


def _run(X, weight, trace=False, **build_kwargs):
    Xb, wb = _prep_inputs(X, weight)
    nc = _build_nc(**build_kwargs)
    nc.compile()
    in_maps = [
        {"x": np.ascontiguousarray(Xb[i * B_LOC : (i + 1) * B_LOC]), "wb": wb}
        for i in range(N_CORES)
    ]
    res = run_bass_kernel_spmd(nc, in_maps, core_ids=list(range(N_CORES)), trace=trace)
    out = np.concatenate([r["out"] for r in res.results], axis=0)
    return out, res


def kernel(X, weight):
    return _run(X, weight)[0]


# revision 52
# speedup vs baseline: 1.3280x; 1.3280x over previous
"""3x3 conv2d (stride 1, pad 1) over [32, 1024, 1024] fp32, data-parallel on 8 TRN2 cores.

Strategy (memory-bound regime):
  - Pure data parallel: each core gets 4 images; no collectives.
  - Host pads each image to [1026, 1026] with zeros and casts to bf16
    (halves the input DMA traffic; rel err ~1e-3 << 2e-2 gate).
  - On device, the 3x3 conv is computed with banded matmuls on the
    TensorEngine: rows go on the partition dim; the 3 row-taps become a
    banded lhsT [K=m+2, M=m] built on host from the runtime weight; the 3
    column-taps become 3 column-shifted matmuls accumulating in PSUM.
  - Per output tile of 126 rows x 512 cols: 3 matmuls (dv = -1, 0, +1)
    into a single-bank PSUM tile (8 banks in flight for deep PE/copy
    pipelining), then PSUM->SBUF copies that also cast fp32->bf16, split
    2:1 between VectorE and ScalarE (the only engines that can read PSUM),
    then DMA out as bf16 (halves output traffic; host upcasts to fp32).
    Input DMAs issue on the SP (sync) HWDGE queue, output DMAs on the
    otherwise-idle Pool engine's SWDGE queues, so the three DMA descriptor
    streams pipeline independently of the compute engines; lhsT is kept 128
    columns wide (full array) so bf16 fast-weight-load stays enabled.

Measured (8 cores, steady state via in-NEFF For_i repeat slope): ~65-70 us
per kernel body (cost model: 55 us); DMA floor for the 17.9 MB/core of
traffic is ~55-58 us at the measured ~335 GB/s practical HBM rate.
End-to-end rel err vs the fp32 reference: ~2.4e-3 (bf16 in + bf16 out).
"""

import numpy as np
import ml_dtypes

import concourse.bacc as bacc
import concourse.mybir as mybir
from concourse.tile import TileContext
from concourse.bass_utils import run_bass_kernel_spmd

B, H, W = 32, 1024, 1024
N_CORES = 8
B_LOC = B // N_CORES
M_TILE = 126  # output rows per tile; K = M + 2 input rows <= 128 partitions


def _build_nc(
    b_loc=B_LOC,
    h=H,
    w=W,
    bufs_x=8,
    bufs_ps=8,
    bufs_o=8,
    copy_engines=("vector", "vector", "scalar"),
    out_dma_split=1,
    copy_full=False,
    psum_per_chunk=True,
    in_dma_engine="sync",
    out_dma_engine="gpsimd",
    repeat=1,
    lhst_full=True,
    dma_alternate=False,
    probe_mode="full",  # "full" | "dma_only" | "no_out" | "no_in" | "in_pe"
    out_bf16=True,
    merge_tail=True,
):
    hp, wp = h + 2, w + 2
    n_row_tiles = (h + M_TILE - 1) // M_TILE
    n_col_chunks = (w + 511) // 512
    psum_w = min(w, 1024)
    tail_m = h - (h // M_TILE) * M_TILE
    if merge_tail and tail_m and b_loc * (tail_m + 2) <= 128:
        wb_cols = 384 + 3 * b_loc * tail_m
    else:
        merge_tail = False
        wb_cols = 384

    out_dt = mybir.dt.bfloat16 if out_bf16 else mybir.dt.float32
    nc = bacc.Bacc("TRN2")
    x_d = nc.dram_tensor("x", [b_loc, hp, wp], mybir.dt.bfloat16, kind="ExternalInput")
    w_d = nc.dram_tensor("wb", [128, wb_cols], mybir.dt.bfloat16, kind="ExternalInput")
    o_d = nc.dram_tensor("out", [b_loc, h, w], out_dt, kind="ExternalOutput")

    with TileContext(nc) as tc:
        with (
            tc.tile_pool(name="wpool", bufs=1) as wpool,
            tc.tile_pool(name="xpool", bufs=bufs_x) as xpool,
            tc.tile_pool(name="pspool", bufs=bufs_ps, space="PSUM") as pspool,
            tc.tile_pool(name="opool", bufs=bufs_o) as opool,
        ):
            in_dma = getattr(nc, in_dma_engine)
            out_dma = getattr(nc, out_dma_engine)
            wt = wpool.tile([128, wb_cols], mybir.dt.bfloat16)
            nc.sync.dma_start(out=wt[:], in_=w_d[:, :])

            import contextlib

            rep_ctx = (
                tc.For_i(0, repeat, 1) if repeat > 1 else contextlib.nullcontext()
            )
            with rep_ctx:
                _emit_body(
                    nc, tc, b_loc, h, w, wp, n_row_tiles, n_col_chunks,
                    xpool, pspool, opool, wt, x_d, o_d,
                    copy_engines, out_dma_split, copy_full, psum_per_chunk,
                    in_dma, out_dma, psum_w, lhst_full, dma_alternate, probe_mode,
                    out_dt, merge_tail, tail_m,
                )
    return nc


def _emit_body(
    nc, tc, b_loc, h, w, wp, n_row_tiles, n_col_chunks,
    xpool, pspool, opool, wt, x_d, o_d,
    copy_engines, out_dma_split, copy_full, psum_per_chunk,
    in_dma, out_dma, psum_w, lhst_full=False, dma_alternate=False,
    probe_mode="full",
    out_dt=mybir.dt.float32,
    merge_tail=False,
    tail_m=0,
):
    do_in = probe_mode in ("full", "dma_only", "no_out", "in_pe")
    do_compute = probe_mode in ("full", "no_out", "no_in", "in_pe")
    do_copy = probe_mode in ("full", "no_out", "no_in")
    do_out = probe_mode in ("full", "dma_only", "no_in")
    import concourse.mybir as mybir

    if True:  # keep indentation shallow
        if True:
            it = 0
            n_body_tiles = (h // M_TILE) if merge_tail else n_row_tiles
            for img in range(b_loc):
                for t in range(n_body_tiles):
                    r0 = t * M_TILE
                    m = min(M_TILE, h - r0)
                    k = m + 2
                    if dma_alternate:
                        in_dma = (nc.sync, nc.scalar)[it % 2]
                        out_dma = (nc.scalar, nc.sync)[it % 2]
                    xt = xpool.tile([128, wp], mybir.dt.bfloat16)
                    if do_in:
                        in_dma.dma_start(out=xt[:k, :], in_=x_d[img, r0 : r0 + k, :])
                    elif do_compute:
                        nc.gpsimd.memset(xt[:k, :], 0)
                    if not psum_per_chunk:
                        ps = pspool.tile([128, psum_w], mybir.dt.float32)
                    ot = (
                        opool.tile([128, w], out_dt, name="ot")
                        if (do_copy or do_out)
                        else None
                    )
                    if not do_copy and do_out:
                        nc.vector.memset(ot[:m, :w], 0)
                    for ci in range(n_col_chunks) if do_compute else []:
                        c0 = ci * 512
                        n = min(512, w - c0)
                        m_mm = 128 if lhst_full else m
                        if psum_per_chunk:
                            ps_c = pspool.tile(
                                [128, 512], mybir.dt.float32, name=f"psc_{it}_{ci}", tag="psc"
                            )
                            dst = ps_c[:m_mm, :n]
                            src = ps_c[:m, :n]
                        else:
                            dst = ps[:m_mm, c0 : c0 + n]
                            src = ps[:m, c0 : c0 + n]
                        for j, dv in enumerate((-1, 0, 1)):
                            nc.tensor.matmul(
                                dst,
                                lhsT=wt[:k, 128 * (dv + 1) : 128 * (dv + 1) + m_mm],
                                rhs=xt[:k, c0 + 1 + dv : c0 + 1 + dv + n],
                                start=(j == 0),
                                stop=(j == 2),
                            )
                        if not copy_full and do_copy:
                            eng = copy_engines[
                                (it * n_col_chunks + ci) % len(copy_engines)
                            ]
                            if eng == "scalar":
                                nc.scalar.copy(ot[:m, c0 : c0 + n], src)
                            else:
                                nc.vector.tensor_copy(ot[:m, c0 : c0 + n], src)
                    if copy_full:
                        eng = copy_engines[it % len(copy_engines)]
                        if eng == "scalar":
                            nc.scalar.copy(ot[:m, :w], ps[:m, :w])
                        else:
                            nc.vector.tensor_copy(ot[:m, :w], ps[:m, :w])
                    if not do_out:
                        pass
                    elif out_dma_split == 1:
                        out_dma.dma_start(out=o_d[img, r0 : r0 + m, :], in_=ot[:m, :w])
                    else:
                        step = w // out_dma_split
                        for s in range(out_dma_split):
                            out_dma.dma_start(
                                out=o_d[img, r0 : r0 + m, s * step : (s + 1) * step],
                                in_=ot[:m, s * step : (s + 1) * step],
                            )
                    it += 1

            if merge_tail:
                # All images' tail rows in one block-diagonal banded matmul:
                # image i occupies partitions [i*(tail_m+2), (i+1)*(tail_m+2))
                # of the input tile and [i*tail_m, (i+1)*tail_m) of the output.
                r0 = (h // M_TILE) * M_TILE
                tk = tail_m + 2
                TK, TM = b_loc * tk, b_loc * tail_m
                xt = xpool.tile([128, wp], mybir.dt.bfloat16, name="xt")
                if do_in:
                    for img in range(b_loc):
                        in_dma.dma_start(
                            out=xt[img * tk : (img + 1) * tk, :],
                            in_=x_d[img, r0 : r0 + tk, :],
                        )
                elif do_compute:
                    nc.gpsimd.memset(xt[:TK, :], 0)
                ot = (
                    opool.tile([128, w], out_dt, name="ot")
                    if (do_copy or do_out)
                    else None
                )
                if not do_copy and do_out:
                    nc.vector.memset(ot[:TM, :w], 0)
                if do_compute and not psum_per_chunk:
                    ps = pspool.tile([128, psum_w], mybir.dt.float32, name="ps")
                for ci in range(n_col_chunks) if do_compute else []:
                    c0 = ci * 512
                    n = min(512, w - c0)
                    if psum_per_chunk:
                        ps_c = pspool.tile(
                            [128, 512], mybir.dt.float32, name="psc_tail", tag="psc"
                        )
                        dst, src = ps_c[:TM, :n], ps_c[:TM, :n]
                    else:
                        dst, src = ps[:TM, c0 : c0 + n], ps[:TM, c0 : c0 + n]
                    for j, dv in enumerate((-1, 0, 1)):
                        nc.tensor.matmul(
                            dst,
                            lhsT=wt[:TK, 384 + (dv + 1) * TM : 384 + (dv + 2) * TM],
                            rhs=xt[:TK, c0 + 1 + dv : c0 + 1 + dv + n],
                            start=(j == 0),
                            stop=(j == 2),
                        )
                    if do_copy:
                        eng = copy_engines[(it * n_col_chunks + ci) % len(copy_engines)]
                        if eng == "scalar":
                            nc.scalar.copy(ot[:TM, c0 : c0 + n], src)
                        else:
                            nc.vector.tensor_copy(ot[:TM, c0 : c0 + n], src)
                if do_out:
                    for img in range(b_loc):
                        out_dma.dma_start(
                            out=o_d[img, r0:h, :],
                            in_=ot[img * tail_m : (img + 1) * tail_m, :w],
                        )


def _banded_weights(weight, b_loc=B_LOC, h=H, merge_tail=True):
    """wb[k, 128*dvi + m] = weight[k - m, dvi] for 0 <= k - m <= 2, else 0.

    When merge_tail, appends per-dv block-diagonal bands [b_loc*(tail_m+2),
    b_loc*tail_m] that compute every image's tail-tile rows in one matmul.
    """
    tail_m = h - (h // M_TILE) * M_TILE
    tw = b_loc * tail_m if (merge_tail and tail_m and b_loc * (tail_m + 2) <= 128) else 0
    wb = np.zeros((128, 384 + 3 * tw), np.float32)
    for dvi in range(3):
        blk = wb[:, 128 * dvi : 128 * dvi + 128]
        for d in range(3):
            rows = np.arange(d, 128)
            cols = np.arange(0, 128 - d)
            blk[rows, cols] = float(weight[d, dvi])
    if tw:
        tk = tail_m + 2
        for dvi in range(3):
            blk = wb[:, 384 + dvi * tw : 384 + (dvi + 1) * tw]
            for i in range(b_loc):
                for d in range(3):
                    rows = np.arange(d, tail_m + d)
                    cols = np.arange(0, tail_m)
                    blk[i * tk + rows, i * tail_m + cols] = float(weight[d, dvi])
    return wb


def _prep_inputs(X, weight):
    X = np.asarray(X, dtype=np.float32)
    weight = np.asarray(weight, dtype=np.float32)
    Xp = np.zeros((X.shape[0], X.shape[1] + 2, X.shape[2] + 2), np.float32)
    Xp[:, 1:-1, 1:-1] = X
    Xb = Xp.astype(ml_dtypes.bfloat16)
    wb = _banded_weights(weight).astype(ml_dtypes.bfloat16)
    return Xb, wb


def _run(X, weight, trace=False, **build_kwargs):
    Xb, wb = _prep_inputs(X, weight)
    nc = _build_nc(**build_kwargs)
    nc.compile()
    in_maps = [
        {"x": np.ascontiguousarray(Xb[i * B_LOC : (i + 1) * B_LOC]), "wb": wb}
        for i in range(N_CORES)
    ]
    res = run_bass_kernel_spmd(nc, in_maps, core_ids=list(range(N_CORES)), trace=trace)
    out = np.concatenate([r["out"] for r in res.results], axis=0)
    if out.dtype != np.float32:
        out = out.astype(np.float32)
    return out, res


def kernel(X, weight):
    return _run(X, weight)[0]
